# revision 38
# baseline (speedup 1.0000x reference)
"""Trainium2 Bass kernel for nn_JointNetwork (RNN-T joint: broadcast-add + 2-layer MLP).

Key insight: the module is fully linear (no activation between the Dense layers):
    out[b,t,u,:] = (enc[b,t]+pred[b,u]) @ W0 @ W1 + b0 @ W1 + b1
                 = E'[b,t,:] + P'[b,u,:]
with E' = enc@W0@W1 + b0@W1 + b1  (shape [B,T,V], small)
     P' = pred@W0@W1              (shape [B,U,V], small)
So the 206-GFLOP einsum collapses to tiny matmuls plus a broadcast-add whose
cost is purely the 512 MB HBM write of the output -> memory roofline
(64 MiB per core / ~358 GB/s ~= 190 us).

Sharding: 8 cores, core c handles b = c//4, t-range [(c%4)*128, (c%4)*128+128).
Each core computes its E' shard + its P' on-chip, then streams the output as
128 tiles [T=128 partitions, V=1024] (PE outer-product broadcast of a P' row
into PSUM, DVE adds the per-partition E' rows, batched 2 MB DMA writes whose
destination is a plain [t, u-slice, v] cut of out -> one contiguous 32 KB
descriptor per partition, 8-deep buffer ring).

The kernel body can be emitted `repeat` times in one NEFF (all semaphore
thresholds offset per iteration; input loads run on the scalar engine's HWDGE
ring so they never queue behind the 64 MB of output writes on the sync ring).
Timing uses a repeated NEFF + a chain of donated-buffer executions so the
reported time is the steady-state per-execution device time, unpolluted by the
~90 ms axon-tunnel dispatch+sync latency (which would otherwise dominate by
~300x). Correctness (`kernel()`) uses repeat=1.

Raw Bass (no TileContext): this container's walrus build rejects instructions
with >1 sync-wait, which TileContext's scheduler emits. All synchronization is
explicit single-wait semaphores.
"""

import os
import sys

if "/opt/trn_rl_repo" not in sys.path:
    sys.path.insert(0, "/opt/trn_rl_repo")

import numpy as np

B, T, U, D, H, V = 2, 512, 128, 512, 512, 1024
NCORES = 8
ROWS = 128          # bt rows per core
G = 4               # rows per output DMA (2 MB per dma_start: ~342 GB/s/core
                    # measured vs ~320 at 4 MB)
NGROUPS = ROWS // G

_cache = {}


def _build_nc(repeat=1, split=4):
    """split = rows per 8-row group drained by DVE (tensor_add of E');
    the other 8-split rows are drained by ScalarE (Copy) after the PE
    pre-accumulates E' into PSUM via an identity matmul. Splitting matters
    because a lone DVE is the pipeline bottleneck: fp32 PSUM-source
    tensor ops run in 1x mode (~1.2 us op + proportional pipe-DRAIN
    ~= 2.1 us effective per 4 KB row -> 270 us/iter on DVE alone, vs the
    ~216 us DMA floor)."""
    import concourse.bass as bass
    import concourse.mybir as mybir
    from contextlib import ExitStack

    fp32 = mybir.dt.float32
    bf16 = mybir.dt.bfloat16
    nc = bass.Bass()

    enc_d = nc.dram_tensor("enc", [ROWS, D], fp32, kind="ExternalInput")
    pred_d = nc.dram_tensor("pred", [U, D], fp32, kind="ExternalInput")
    w0_d = nc.dram_tensor("w0", [D, H], fp32, kind="ExternalInput")
    w1_d = nc.dram_tensor("w1", [H, V], fp32, kind="ExternalInput")
    b0_d = nc.dram_tensor("b0", [H], fp32, kind="ExternalInput")
    b1_d = nc.dram_tensor("b1", [V], fp32, kind="ExternalInput")
    out_d = nc.dram_tensor("out", [ROWS, U, V], fp32, kind="ExternalOutput")

    KD = D // 128   # 4 contraction blocks over d
    KH = H // 128   # 4 contraction blocks over h
    NV = V // 512   # 2 moving-dim chunks over v

    # per-iteration semaphore increments
    DMAI = 96       # 6 input loads x 16
    PREP = 18
    CP = 21         # 18 phase-A publishes + Phi/Pf/Plo bf16-split of P'
    PD = ROWS       # pe_done
    DOUT = 16 * NGROUPS
    S = split
    DVE_I = NGROUPS * S            # dve_done per iter
    ACT_I = NGROUPS * (G - S)      # act_done per iter

    def is_dve(i):
        return (i % G) < S

    def ndve(j):
        # DVE-drained rows among rows 0..j
        return (j // G) * S + min(j % G + 1, S)

    def nact(j):
        return (j // G) * (G - S) + max(j % G + 1 - S, 0)

    with ExitStack() as st:
        def sb(name, shape):
            return st.enter_context(nc.sbuf_tensor(name, shape, fp32))

        enc_s = sb("enc_s", [128, D])
        pred_s = sb("pred_s", [128, D])
        w0_s = sb("w0_s", [128, KD, H])        # w0_s[p,k,h] = W0[k*128+p, h]
        w1_s = sb("w1_s", [128, KH, V])        # w1_s[p,k,v] = W1[k*128+p, v]
        b0t_s = sb("b0t_s", [128, KH])         # b0t_s[p,k]  = b0[k*128+p]
        b1_s = sb("b1_s", [1, V])
        ones_s = sb("ones_s", [1, 128])
        ident_s = sb("ident_s", [128, 128])
        encT_s = sb("encT_s", [128, KD, 128])  # encT_s[p,k,j] = enc[j, k*128+p]
        predT_s = sb("predT_s", [128, KD, 128])
        e1t_s = sb("e1t_s", [128, KH, 128])    # e1t[p,k,j] = (enc@W0+b0)[j, k*128+p]
        p1t_s = sb("p1t_s", [128, KH, 128])
        E_s = sb("E_s", [128, V])              # E'[bt, v]
        P_s = sb("P_s", [128, V])              # P'[u, v]
        # exact two-term bf16 split of P' (P' = Phi + Plo + O(|P'| 2^-17));
        # lets phase B broadcast P' rows with bf16 matmuls (1 cyc/col) instead
        # of fp32 ones (4 cyc/col), which would make the PE the bottleneck
        identb_s = st.enter_context(nc.sbuf_tensor("identb_s", [128, 128], bf16))
        Phi_s = st.enter_context(nc.sbuf_tensor("Phi_s", [128, V], bf16))
        Plo_s = st.enter_context(nc.sbuf_tensor("Plo_s", [128, V], bf16))
        Pf_s = sb("Pf_s", [128, V])            # fp32 upcast of Phi
        obuf = [sb(f"obuf{i}", [128, G, V]) for i in range(8)]
        psum = [
            st.enter_context(nc.psum_tensor(f"ps{i}", [128, V], fp32))
            for i in range(4)
        ]

        dma_sem = st.enter_context(nc.semaphore("dma_in"))
        g_sem = st.enter_context(nc.semaphore("gsim"))
        pe_prep = st.enter_context(nc.semaphore("pe_prep"))
        cp_sem = st.enter_context(nc.semaphore("cp"))
        pe_done = st.enter_context(nc.semaphore("pe_done"))
        dve_done = st.enter_context(nc.semaphore("dve_done"))
        act_done = st.enter_context(nc.semaphore("act_done"))
        dma_out = st.enter_context(nc.semaphore("dma_out"))

        blk = st.enter_context(nc.Block())

        @blk.gpsimd
        def _(g):
            g.memset(ones_s[:], 1.0)
            g.memset(ident_s[:], 0.0)
            g.affine_select(
                out=ident_s[:], in_=ident_s[:],
                compare_op=mybir.AluOpType.not_equal,
                fill=1.0, base=0, pattern=[[-1, 128]], channel_multiplier=1,
            )
            g.memset(identb_s[:], 0.0)
            g.affine_select(
                out=identb_s[:], in_=identb_s[:],
                compare_op=mybir.AluOpType.not_equal,
                fill=1.0, base=0, pattern=[[-1, 128]], channel_multiplier=1,
            ).then_inc(g_sem, 1)
            # input loads on the SWDGE ring: GpSimd is otherwise idle, and
            # this keeps the loads off both the sync ring (they would queue
            # FIFO behind 64 MB of output writes) and ScalarE (whose phase-B
            # copies must not serialize against next-iter loads).
            for k in range(repeat):
                if k > 0:
                    # input-tile consumers of iter k-1 are done once DVE's
                    # first phase-B op of iter k-1 has retired
                    g.wait_ge(dve_done, DVE_I * (k - 1) + 1)
                g.dma_start(enc_s[:], enc_d[:]).then_inc(dma_sem, 16)
                g.dma_start(pred_s[:], pred_d[:]).then_inc(dma_sem, 16)
                g.dma_start(w0_s[:], w0_d[:].rearrange("(k p) h -> p k h", p=128)).then_inc(dma_sem, 16)
                g.dma_start(w1_s[:], w1_d[:].rearrange("(k p) v -> p k v", p=128)).then_inc(dma_sem, 16)
                with nc.allow_non_contiguous_dma(reason="tiny 2KB b0 transpose load"):
                    g.dma_start(b0t_s[:], b0_d[:].rearrange("(k p) -> p k", p=128)).then_inc(dma_sem, 16)
                g.dma_start(b1_s[:], b1_d[None, :]).then_inc(dma_sem, 16)

        if S < G:
            @blk.scalar
            def _(sc):
                # ScalarE drains its share of phase-B rows (PSUM -> obuf Copy;
                # the PE already accumulated E' into those rows).
                for k in range(repeat):
                    for i in range(ROWS):
                        if is_dve(i):
                            continue
                        g_abs = NGROUPS * k + i // G
                        if i % G == S and g_abs >= 4:
                            sc.wait_ge(dma_out, 16 * (g_abs - 3))
                        sc.wait_ge(pe_done, PD * k + i + 1)
                        sc.activation(
                            obuf[(i // G) % 4][:, i % G, :], psum[i % 4][:],
                            mybir.ActivationFunctionType.Copy,
                        ).then_inc(act_done, 1)

        @blk.sync
        def _(s):
            for k in range(repeat):
                for g in range(NGROUPS):
                    g_abs = NGROUPS * k + g
                    if S > 0:
                        s.wait_ge(dve_done, DVE_I * k + S * (g + 1))
                    if S < G:
                        s.wait_ge(act_done, ACT_I * k + (G - S) * (g + 1))
                    # dest is a plain slice: per partition t, one contiguous
                    # 32 KB descriptor (G*V floats) at stride U*V*4
                    s.dma_start(
                        out_d[:, g * G:(g + 1) * G, :], obuf[g % 8][:]
                    ).then_inc(dma_out, 16)
            s.wait_ge(dma_out, DOUT * repeat)

        @blk.tensor
        def _(pe):
            for k in range(repeat):
                pe.wait_ge(dma_sem, DMAI * (k + 1))
                if k == 0:
                    pe.wait_ge(g_sem, 1)
                else:
                    # all psum readers (phase-B drains) of iter k-1 retired
                    if S > 0:
                        pe.wait_ge(dve_done, DVE_I * k)
                    if S < G:
                        pe.wait_ge(act_done, ACT_I * k)
                # --- transposes of enc (j=0..3) and pred (j=4..7) into bank0 of psum[j%2]
                srcs = [(enc_s, kk) for kk in range(KD)] + [(pred_s, kk) for kk in range(KD)]
                for j, (src, kk) in enumerate(srcs):
                    if j >= 2:
                        pe.wait_ge(cp_sem, CP * k + j - 1)
                    pe.transpose(
                        psum[j % 2][:, 0:128], src[:, kk * 128:(kk + 1) * 128], ident_s[:]
                    ).then_inc(pe_prep, 1)                       # pe_prep 1..8
                # --- E1T = (W0^T blocks) @ encT, accumulated over d-blocks
                for hb in range(KH):
                    if hb >= 2:
                        pe.wait_ge(cp_sem, CP * k + 7 + hb)
                    for kk in range(KD):
                        ins = pe.matmul(
                            psum[2 + hb % 2][:, 0:128],
                            w0_s[:, kk, hb * 128:(hb + 1) * 128],
                            encT_s[:, kk, :],
                            start=(kk == 0), stop=(kk == KD - 1),
                        )
                    ins.then_inc(pe_prep, 1)                     # pe_prep 9..12
                # --- P1T, bank1 of psum[2]/psum[3]
                for hb in range(KH):
                    if hb >= 2:
                        pe.wait_ge(cp_sem, CP * k + 11 + hb)
                    for kk in range(KD):
                        ins = pe.matmul(
                            psum[2 + hb % 2][:, 512:640],
                            w0_s[:, kk, hb * 128:(hb + 1) * 128],
                            predT_s[:, kk, :],
                            start=(kk == 0), stop=(kk == KD - 1),
                        )
                    ins.then_inc(pe_prep, 1)                     # pe_prep 13..16
                # --- E' = E1^T^T @ W1 + ones^T @ b1 -> psum[0] (both banks)
                pe.wait_ge(cp_sem, CP * k + 7)
                for vc in range(NV):
                    for hb in range(KH):
                        pe.matmul(
                            psum[0][:, vc * 512:(vc + 1) * 512],
                            e1t_s[:, hb, :],
                            w1_s[:, hb, vc * 512:(vc + 1) * 512],
                            start=(hb == 0), stop=False,
                        )
                    ins = pe.matmul(
                        psum[0][:, vc * 512:(vc + 1) * 512],
                        ones_s[:],
                        b1_s[0:1, vc * 512:(vc + 1) * 512],
                        start=False, stop=True,
                    )
                ins.then_inc(pe_prep, 1)                         # pe_prep 17
                # --- P' -> psum[1]
                pe.wait_ge(cp_sem, CP * k + 8)
                for vc in range(NV):
                    for hb in range(KH):
                        ins = pe.matmul(
                            psum[1][:, vc * 512:(vc + 1) * 512],
                            p1t_s[:, hb, :],
                            w1_s[:, hb, vc * 512:(vc + 1) * 512],
                            start=(hb == 0), stop=(hb == KH - 1),
                        )
                ins.then_inc(pe_prep, 1)                         # pe_prep 18
                # --- phase B: broadcast each P' row (u=i) across the 128
                # t-partitions via TWO accumulating bf16 matmuls (Phi + Plo,
                # an exact two-term split of P' -- bf16 streams 1 col/cycle
                # vs fp32's 4). DVE-drained rows then get E' added by DVE;
                # for ScalarE-drained rows the PE also accumulates E' (fp32)
                # so ScalarE only needs a Copy.
                pe.wait_ge(cp_sem, CP * k + 21)
                for i in range(ROWS):
                    if i >= 4:
                        j = i - 4   # previous reader of psum[i % 4]
                        if is_dve(j):
                            pe.wait_ge(dve_done, DVE_I * k + ndve(j))
                        else:
                            pe.wait_ge(act_done, ACT_I * k + nact(j))
                    # selb = e_i ⊗ ones: out[t,v] = sum_c δ(c,i)·X[c,v] = X[i,v] ∀t
                    selb = identb_s[:, i:i + 1].broadcast_to([128, 128])
                    for vc in range(NV):
                        pe.matmul(
                            psum[i % 4][:, vc * 512:(vc + 1) * 512],
                            selb,
                            Phi_s[:, vc * 512:(vc + 1) * 512],
                            start=True, stop=False,
                        )
                    for vc in range(NV):
                        ins = pe.matmul(
                            psum[i % 4][:, vc * 512:(vc + 1) * 512],
                            selb,
                            Plo_s[:, vc * 512:(vc + 1) * 512],
                            start=False, stop=is_dve(i),
                        )
                    if not is_dve(i):
                        for vc in range(NV):
                            ins = pe.matmul(
                                psum[i % 4][:, vc * 512:(vc + 1) * 512],
                                ident_s[:],
                                E_s[:, vc * 512:(vc + 1) * 512],
                                start=False, stop=True,
                            )
                    ins.then_inc(pe_done, 1)

        @blk.vector
        def _(v):
            for k in range(repeat):
                # copies for the 8 transposes
                dsts = [(encT_s, kk) for kk in range(KD)] + [(predT_s, kk) for kk in range(KD)]
                for j, (dst, kk) in enumerate(dsts):
                    v.wait_ge(pe_prep, PREP * k + j + 1)
                    v.tensor_copy(dst[:, kk, :], psum[j % 2][:, 0:128]).then_inc(cp_sem, 1)
                for hb in range(KH):                             # e1t + bias b0
                    v.wait_ge(pe_prep, PREP * k + 9 + hb)
                    v.tensor_scalar_add(
                        e1t_s[:, hb, :], psum[2 + hb % 2][:, 0:128], b0t_s[:, hb:hb + 1]
                    ).then_inc(cp_sem, 1)
                for hb in range(KH):                             # p1t
                    v.wait_ge(pe_prep, PREP * k + 13 + hb)
                    v.tensor_copy(
                        p1t_s[:, hb, :], psum[2 + hb % 2][:, 512:640]
                    ).then_inc(cp_sem, 1)
                v.wait_ge(pe_prep, PREP * k + 17)
                v.tensor_copy(E_s[:], psum[0][:]).then_inc(cp_sem, 1)
                v.wait_ge(pe_prep, PREP * k + 18)
                v.tensor_copy(P_s[:], psum[1][:]).then_inc(cp_sem, 1)
                # exact bf16 split: Phi = bf16(P'), Plo = bf16(P' - fp32(Phi))
                v.tensor_copy(Phi_s[:], P_s[:]).then_inc(cp_sem, 1)       # 19
                v.tensor_copy(Pf_s[:], Phi_s[:]).then_inc(cp_sem, 1)      # 20
                v.tensor_sub(Plo_s[:], P_s[:], Pf_s[:]).then_inc(cp_sem, 1)  # 21
                # --- DVE share of phase B (add E' while draining PSUM)
                for i in range(ROWS):
                    if not is_dve(i):
                        continue
                    g_abs = NGROUPS * k + i // G
                    if i % G == 0 and g_abs >= 8:
                        # obuf[g%8] was last read by output-DMA group g_abs-8.
                        # dma_out counts PER-ENGINE completions (16 SDMA
                        # engines x 1 inc per dma_start), so count >= 16*m
                        # does NOT imply the first m groups are fully done
                        # when engines skew. Wait 2 groups of count-margin
                        # (threshold g_abs-5, i.e. 5-group production lead):
                        # tolerates ~2 groups (~13 us) of engine skew, same
                        # absolute tolerance as the proven 4 MB config.
                        v.wait_ge(dma_out, 16 * (g_abs - 5))
                    v.wait_ge(pe_done, PD * k + i + 1)
                    v.tensor_add(
                        obuf[(i // G) % 8][:, i % G, :], psum[i % 4][:], E_s[:]
                    ).then_inc(dve_done, 1)

    return nc


def _in_maps(pred_inp, enc_inp, W0, b0, W1, b1):
    maps = []
    for c in range(NCORES):
        b = c // 4
        t0 = (c % 4) * ROWS
        maps.append({
            "enc": np.ascontiguousarray(enc_inp[b, t0:t0 + ROWS, :], dtype=np.float32),
            "pred": np.ascontiguousarray(pred_inp[b], dtype=np.float32),
            "w0": np.ascontiguousarray(W0, dtype=np.float32),
            "w1": np.ascontiguousarray(W1, dtype=np.float32),
            "b0": np.ascontiguousarray(b0, dtype=np.float32),
            "b1": np.ascontiguousarray(b1, dtype=np.float32),
        })
    return maps


def _run(pred_inp, enc_inp, W0, b0, W1, b1, trace=False):
    from concourse.bass_utils import run_bass_kernel_spmd

    if "nc" not in _cache:
        _cache["nc"] = _build_nc(repeat=1)
    nc = _cache["nc"]
    res = run_bass_kernel_spmd(
        nc, _in_maps(pred_inp, enc_inp, W0, b0, W1, b1),
        list(range(NCORES)), trace=trace,
    )
    out = np.empty((B, T, U, V), dtype=np.float32)
    for c in range(NCORES):
        b = c // 4
        t0 = (c % 4) * ROWS
        out[b, t0:t0 + ROWS] = res.results[c]["out"]
    return out, res


def kernel(pred_inp, enc_inp, W0, b0, W1, b1):
    out, _ = _run(pred_inp, enc_inp, W0, b0, W1, b1, trace=False)
    return out


def _setup_timed(nc):
    """jit + fast-dispatch-compile the 8-core shard_map execution of `nc`."""
    import jax
    from concourse import bass2jax, mybir

    bass2jax.install_neuronx_cc_hook()
    in_names, out_names, out_avals, zero_outs = [], [], [], []
    pname = nc.partition_id_tensor.name if nc.partition_id_tensor else None
    for alloc in nc.m.functions[0].allocations:
        if not isinstance(alloc, mybir.MemoryLocationSet):
            continue
        name = alloc.memorylocations[0].name
        if alloc.kind == "ExternalInput":
            if name != pname:
                in_names.append(name)
        elif alloc.kind == "ExternalOutput":
            out_names.append(name)
            shape = tuple(alloc.tensor_shape)
            dt = mybir.dt.np(alloc.dtype)
            out_avals.append(jax.core.ShapedArray(shape, dt))
            zero_outs.append(np.zeros(shape, dt))
    n_params = len(in_names)
    all_names = in_names + out_names
    if pname is not None:
        all_names = all_names + [pname]

    def _body(*args):
        operands = list(args)
        if pname is not None:
            operands.append(bass2jax.partition_id_tensor())
        outs = bass2jax._bass_exec_p.bind(
            *operands,
            out_avals=tuple(out_avals),
            in_names=tuple(all_names),
            out_names=tuple(out_names),
            lowering_input_output_aliases=(),
            sim_require_finite=True,
            sim_require_nnan=True,
            nc=nc,
        )
        return tuple(outs)

    devices = jax.devices()[:NCORES]
    mesh = bass2jax.Mesh(np.asarray(devices), ("core",))
    P = bass2jax.PartitionSpec("core")
    donate = tuple(range(n_params, n_params + len(out_names)))
    jitted = jax.jit(
        bass2jax.shard_map(
            _body, mesh=mesh, in_specs=(P,) * (n_params + len(out_names)),
            out_specs=(P,) * len(out_names), check_rep=False,
        ),
        donate_argnums=donate, keep_unused=True,
    )
    sh = jax.sharding.NamedSharding(mesh, P)
    return jitted, in_names, zero_outs, sh


TIMED_REPEAT = 96     # kernel-body repetitions inside the timed NEFF
TIMED_CHAIN = 512     # executions chained per measured sync
TIMED_BATCHES = 3


def _timed_run(pred_inp, enc_inp, W0, b0, W1, b1, iters=None):
    """Steady-state on-device timing (no NTFF hook in this container).

    The timed NEFF contains TIMED_REPEAT back-to-back emissions of the full
    kernel body (loads + E'/P' + output streaming); TIMED_CHAIN executions of
    it are chained through donated output buffers and synced once. Reported
    time = wall / (TIMED_CHAIN * TIMED_REPEAT): the steady-state time of one
    full kernel execution on the hardware. A single non-amortized dispatch
    would instead measure the ~90 ms axon-tunnel round-trip, ~300x the
    kernel's actual device time.
    Returns (full_output, best_exec_ns).
    """
    import time
    import jax
    from concourse import bass2jax

    if "nc_t" not in _cache:
        _cache["nc_t"] = _build_nc(repeat=TIMED_REPEAT)
    nc = _cache["nc_t"]
    jitted, in_names, zero_outs, sh = _setup_timed(nc)

    maps = _in_maps(pred_inp, enc_inp, W0, b0, W1, b1)
    concat_in = [
        jax.device_put(
            np.concatenate([maps[c][nm] for c in range(NCORES)], axis=0), sh
        )
        for nm in in_names
    ]
    jax.block_until_ready(concat_in)
    d_zeros = [
        jax.device_put(
            np.zeros((NCORES * z.shape[0], *z.shape[1:]), z.dtype), sh
        )
        for z in zero_outs
    ]
    jax.block_until_ready(d_zeros)

    fast = bass2jax.fast_dispatch_compile(
        lambda: jitted.lower(*concat_in, *d_zeros).compile()
    )
    outs = fast(*concat_in, *d_zeros)
    jax.block_until_ready(outs)

    best = None
    for it in range(TIMED_BATCHES):
        t0 = time.perf_counter()
        for _ in range(TIMED_CHAIN):
            outs = fast(*concat_in, *outs)
        jax.block_until_ready(outs)
        dt_ns = (time.perf_counter() - t0) * 1e9 / (TIMED_CHAIN * TIMED_REPEAT)
        if os.environ.get("TIME_DEBUG"):
            print(f"  batch {it}: {dt_ns/1e3:.1f} us/exec")
        best = dt_ns if best is None else min(best, dt_ns)

    res0 = np.asarray(outs[0]).reshape(NCORES, ROWS, U, V)
    full = np.empty((B, T, U, V), dtype=np.float32)
    for c in range(NCORES):
        b = c // 4
        t0_ = (c % 4) * ROWS
        full[b, t0_:t0_ + ROWS] = res0[c]
    return full, int(best)


# revision 39
# speedup vs baseline: 1.0116x; 1.0116x over previous
"""Trainium2 Bass kernel for nn_JointNetwork (RNN-T joint: broadcast-add + 2-layer MLP).

Key insight: the module is fully linear (no activation between the Dense layers):
    out[b,t,u,:] = (enc[b,t]+pred[b,u]) @ W0 @ W1 + b0 @ W1 + b1
                 = E'[b,t,:] + P'[b,u,:]
with E' = enc@W0@W1 + b0@W1 + b1  (shape [B,T,V], small)
     P' = pred@W0@W1              (shape [B,U,V], small)
So the 206-GFLOP einsum collapses to tiny matmuls plus a broadcast-add whose
cost is purely the 512 MB HBM write of the output -> memory roofline
(64 MiB per core / ~358 GB/s ~= 190 us).

Sharding: 8 cores, core c handles b = c//4, t-range [(c%4)*128, (c%4)*128+128).
Each core computes its E' shard + its P' on-chip, then streams the output as
128 tiles [T=128 partitions, V=1024] (PE outer-product broadcast of a P' row
into PSUM, DVE adds the per-partition E' rows, batched 4 MB DMA writes whose
destination is a plain [t, u-slice, v] cut of out -> one contiguous 32 KB
descriptor per partition, 4-deep buffer ring).

The kernel body can be emitted `repeat` times in one NEFF (all semaphore
thresholds offset per iteration; input loads run on the scalar engine's HWDGE
ring so they never queue behind the 64 MB of output writes on the sync ring).
Timing uses a repeated NEFF + a chain of donated-buffer executions so the
reported time is the steady-state per-execution device time, unpolluted by the
~90 ms axon-tunnel dispatch+sync latency (which would otherwise dominate by
~300x). Correctness (`kernel()`) uses repeat=1.

Raw Bass (no TileContext): this container's walrus build rejects instructions
with >1 sync-wait, which TileContext's scheduler emits. All synchronization is
explicit single-wait semaphores.
"""

import os
import sys

if "/opt/trn_rl_repo" not in sys.path:
    sys.path.insert(0, "/opt/trn_rl_repo")

import numpy as np

B, T, U, D, H, V = 2, 512, 128, 512, 512, 1024
NCORES = 8
ROWS = 128          # bt rows per core
G = 8               # rows per output DMA (4 MB per dma_start)
NGROUPS = ROWS // G

_cache = {}


def _build_nc(repeat=1, split=8):
    """split = rows per 8-row group drained by DVE (tensor_add of E');
    the other 8-split rows are drained by ScalarE (Copy) after the PE
    pre-accumulates E' into PSUM via an identity matmul. Splitting matters
    because a lone DVE is the pipeline bottleneck: fp32 PSUM-source
    tensor ops run in 1x mode (~1.2 us op + proportional pipe-DRAIN
    ~= 2.1 us effective per 4 KB row -> 270 us/iter on DVE alone, vs the
    ~216 us DMA floor)."""
    import concourse.bass as bass
    import concourse.mybir as mybir
    from contextlib import ExitStack

    fp32 = mybir.dt.float32
    bf16 = mybir.dt.bfloat16
    nc = bass.Bass()

    enc_d = nc.dram_tensor("enc", [ROWS, D], fp32, kind="ExternalInput")
    pred_d = nc.dram_tensor("pred", [U, D], fp32, kind="ExternalInput")
    w0_d = nc.dram_tensor("w0", [D, H], fp32, kind="ExternalInput")
    w1_d = nc.dram_tensor("w1", [H, V], fp32, kind="ExternalInput")
    b0_d = nc.dram_tensor("b0", [H], fp32, kind="ExternalInput")
    b1_d = nc.dram_tensor("b1", [V], fp32, kind="ExternalInput")
    out_d = nc.dram_tensor("out", [ROWS, U, V], fp32, kind="ExternalOutput")

    KD = D // 128   # 4 contraction blocks over d
    KH = H // 128   # 4 contraction blocks over h
    NV = V // 512   # 2 moving-dim chunks over v

    # per-iteration semaphore increments
    DMAI = 96       # 6 input loads x 16
    PREP = 18
    CP = 21         # 18 phase-A publishes + Phi/Pf/Plo bf16-split of P'
    PD = ROWS       # pe_done
    DOUT = 16 * NGROUPS
    S = split
    DVE_I = NGROUPS * S            # dve_done per iter
    ACT_I = NGROUPS * (G - S)      # act_done per iter

    def is_dve(i):
        return (i % G) < S

    def ndve(j):
        # DVE-drained rows among rows 0..j
        return (j // G) * S + min(j % G + 1, S)

    def nact(j):
        return (j // G) * (G - S) + max(j % G + 1 - S, 0)

    with ExitStack() as st:
        def sb(name, shape):
            return st.enter_context(nc.sbuf_tensor(name, shape, fp32))

        enc_s = sb("enc_s", [128, D])
        pred_s = sb("pred_s", [128, D])
        w0_s = sb("w0_s", [128, KD, H])        # w0_s[p,k,h] = W0[k*128+p, h]
        w1_s = sb("w1_s", [128, KH, V])        # w1_s[p,k,v] = W1[k*128+p, v]
        b0t_s = sb("b0t_s", [128, KH])         # b0t_s[p,k]  = b0[k*128+p]
        b1_s = sb("b1_s", [1, V])
        ones_s = sb("ones_s", [1, 128])
        ident_s = sb("ident_s", [128, 128])
        encT_s = sb("encT_s", [128, KD, 128])  # encT_s[p,k,j] = enc[j, k*128+p]
        predT_s = sb("predT_s", [128, KD, 128])
        e1t_s = sb("e1t_s", [128, KH, 128])    # e1t[p,k,j] = (enc@W0+b0)[j, k*128+p]
        p1t_s = sb("p1t_s", [128, KH, 128])
        E_s = sb("E_s", [128, V])              # E'[bt, v]
        P_s = sb("P_s", [128, V])              # P'[u, v]
        # exact two-term bf16 split of P' (P' = Phi + Plo + O(|P'| 2^-17));
        # lets phase B broadcast P' rows with bf16 matmuls (1 cyc/col) instead
        # of fp32 ones (4 cyc/col), which would make the PE the bottleneck
        identb_s = st.enter_context(nc.sbuf_tensor("identb_s", [128, 128], bf16))
        Phi_s = st.enter_context(nc.sbuf_tensor("Phi_s", [128, V], bf16))
        Plo_s = st.enter_context(nc.sbuf_tensor("Plo_s", [128, V], bf16))
        Pf_s = sb("Pf_s", [128, V])            # fp32 upcast of Phi
        obuf = [sb(f"obuf{i}", [128, G, V]) for i in range(4)]
        psum = [
            st.enter_context(nc.psum_tensor(f"ps{i}", [128, V], fp32))
            for i in range(4)
        ]

        dma_sem = st.enter_context(nc.semaphore("dma_in"))
        g_sem = st.enter_context(nc.semaphore("gsim"))
        pe_prep = st.enter_context(nc.semaphore("pe_prep"))
        cp_sem = st.enter_context(nc.semaphore("cp"))
        pe_done = st.enter_context(nc.semaphore("pe_done"))
        dve_done = st.enter_context(nc.semaphore("dve_done"))
        act_done = st.enter_context(nc.semaphore("act_done"))
        dma_out = st.enter_context(nc.semaphore("dma_out"))

        blk = st.enter_context(nc.Block())

        @blk.gpsimd
        def _(g):
            g.memset(ones_s[:], 1.0)
            g.memset(ident_s[:], 0.0)
            g.affine_select(
                out=ident_s[:], in_=ident_s[:],
                compare_op=mybir.AluOpType.not_equal,
                fill=1.0, base=0, pattern=[[-1, 128]], channel_multiplier=1,
            )
            g.memset(identb_s[:], 0.0)
            g.affine_select(
                out=identb_s[:], in_=identb_s[:],
                compare_op=mybir.AluOpType.not_equal,
                fill=1.0, base=0, pattern=[[-1, 128]], channel_multiplier=1,
            ).then_inc(g_sem, 1)
            # input loads on the SWDGE ring: GpSimd is otherwise idle, and
            # this keeps the loads off both the sync ring (they would queue
            # FIFO behind 64 MB of output writes) and ScalarE (whose phase-B
            # copies must not serialize against next-iter loads).
            for k in range(repeat):
                if k > 0:
                    # input-tile consumers of iter k-1 are done once DVE's
                    # first phase-B op of iter k-1 has retired
                    g.wait_ge(dve_done, DVE_I * (k - 1) + 1)
                g.dma_start(enc_s[:], enc_d[:]).then_inc(dma_sem, 16)
                g.dma_start(pred_s[:], pred_d[:]).then_inc(dma_sem, 16)
                g.dma_start(w0_s[:], w0_d[:].rearrange("(k p) h -> p k h", p=128)).then_inc(dma_sem, 16)
                g.dma_start(w1_s[:], w1_d[:].rearrange("(k p) v -> p k v", p=128)).then_inc(dma_sem, 16)
                with nc.allow_non_contiguous_dma(reason="tiny 2KB b0 transpose load"):
                    g.dma_start(b0t_s[:], b0_d[:].rearrange("(k p) -> p k", p=128)).then_inc(dma_sem, 16)
                g.dma_start(b1_s[:], b1_d[None, :]).then_inc(dma_sem, 16)

        if S < G:
            @blk.scalar
            def _(sc):
                # ScalarE drains its share of phase-B rows (PSUM -> obuf Copy;
                # the PE already accumulated E' into those rows).
                for k in range(repeat):
                    for i in range(ROWS):
                        if is_dve(i):
                            continue
                        g_abs = NGROUPS * k + i // G
                        if i % G == S and g_abs >= 4:
                            sc.wait_ge(dma_out, 16 * (g_abs - 3))
                        sc.wait_ge(pe_done, PD * k + i + 1)
                        sc.activation(
                            obuf[(i // G) % 4][:, i % G, :], psum[i % 4][:],
                            mybir.ActivationFunctionType.Copy,
                        ).then_inc(act_done, 1)

        @blk.sync
        def _(s):
            for k in range(repeat):
                for g in range(NGROUPS):
                    g_abs = NGROUPS * k + g
                    if S > 0:
                        s.wait_ge(dve_done, DVE_I * k + S * (g + 1))
                    if S < G:
                        s.wait_ge(act_done, ACT_I * k + (G - S) * (g + 1))
                    # dest is a plain slice: per partition t, one contiguous
                    # 32 KB descriptor (G*V floats) at stride U*V*4
                    s.dma_start(
                        out_d[:, g * G:(g + 1) * G, :], obuf[g % 4][:]
                    ).then_inc(dma_out, 16)
            s.wait_ge(dma_out, DOUT * repeat)

        @blk.tensor
        def _(pe):
            for k in range(repeat):
                pe.wait_ge(dma_sem, DMAI * (k + 1))
                if k == 0:
                    pe.wait_ge(g_sem, 1)
                else:
                    # all psum readers (phase-B drains) of iter k-1 retired
                    if S > 0:
                        pe.wait_ge(dve_done, DVE_I * k)
                    if S < G:
                        pe.wait_ge(act_done, ACT_I * k)
                # --- transposes of enc (j=0..3) and pred (j=4..7) into bank0 of psum[j%2]
                srcs = [(enc_s, kk) for kk in range(KD)] + [(pred_s, kk) for kk in range(KD)]
                for j, (src, kk) in enumerate(srcs):
                    if j >= 2:
                        pe.wait_ge(cp_sem, CP * k + j - 1)
                    pe.transpose(
                        psum[j % 2][:, 0:128], src[:, kk * 128:(kk + 1) * 128], ident_s[:]
                    ).then_inc(pe_prep, 1)                       # pe_prep 1..8
                # --- E1T = (W0^T blocks) @ encT, accumulated over d-blocks
                for hb in range(KH):
                    if hb >= 2:
                        pe.wait_ge(cp_sem, CP * k + 7 + hb)
                    for kk in range(KD):
                        ins = pe.matmul(
                            psum[2 + hb % 2][:, 0:128],
                            w0_s[:, kk, hb * 128:(hb + 1) * 128],
                            encT_s[:, kk, :],
                            start=(kk == 0), stop=(kk == KD - 1),
                        )
                    ins.then_inc(pe_prep, 1)                     # pe_prep 9..12
                # --- P1T, bank1 of psum[2]/psum[3]
                for hb in range(KH):
                    if hb >= 2:
                        pe.wait_ge(cp_sem, CP * k + 11 + hb)
                    for kk in range(KD):
                        ins = pe.matmul(
                            psum[2 + hb % 2][:, 512:640],
                            w0_s[:, kk, hb * 128:(hb + 1) * 128],
                            predT_s[:, kk, :],
                            start=(kk == 0), stop=(kk == KD - 1),
                        )
                    ins.then_inc(pe_prep, 1)                     # pe_prep 13..16
                # --- E' = E1^T^T @ W1 + ones^T @ b1 -> psum[0] (both banks)
                pe.wait_ge(cp_sem, CP * k + 7)
                for vc in range(NV):
                    for hb in range(KH):
                        pe.matmul(
                            psum[0][:, vc * 512:(vc + 1) * 512],
                            e1t_s[:, hb, :],
                            w1_s[:, hb, vc * 512:(vc + 1) * 512],
                            start=(hb == 0), stop=False,
                        )
                    ins = pe.matmul(
                        psum[0][:, vc * 512:(vc + 1) * 512],
                        ones_s[:],
                        b1_s[0:1, vc * 512:(vc + 1) * 512],
                        start=False, stop=True,
                    )
                ins.then_inc(pe_prep, 1)                         # pe_prep 17
                # --- P' -> psum[1]
                pe.wait_ge(cp_sem, CP * k + 8)
                for vc in range(NV):
                    for hb in range(KH):
                        ins = pe.matmul(
                            psum[1][:, vc * 512:(vc + 1) * 512],
                            p1t_s[:, hb, :],
                            w1_s[:, hb, vc * 512:(vc + 1) * 512],
                            start=(hb == 0), stop=(hb == KH - 1),
                        )
                ins.then_inc(pe_prep, 1)                         # pe_prep 18
                # --- phase B: broadcast each P' row (u=i) across the 128
                # t-partitions via TWO accumulating bf16 matmuls (Phi + Plo,
                # an exact two-term split of P' -- bf16 streams 1 col/cycle
                # vs fp32's 4). DVE-drained rows then get E' added by DVE;
                # for ScalarE-drained rows the PE also accumulates E' (fp32)
                # so ScalarE only needs a Copy.
                pe.wait_ge(cp_sem, CP * k + 21)
                for i in range(ROWS):
                    if i >= 4:
                        j = i - 4   # previous reader of psum[i % 4]
                        if is_dve(j):
                            pe.wait_ge(dve_done, DVE_I * k + ndve(j))
                        else:
                            pe.wait_ge(act_done, ACT_I * k + nact(j))
                    # selb = e_i ⊗ ones: out[t,v] = sum_c δ(c,i)·X[c,v] = X[i,v] ∀t
                    selb = identb_s[:, i:i + 1].broadcast_to([128, 128])
                    for vc in range(NV):
                        pe.matmul(
                            psum[i % 4][:, vc * 512:(vc + 1) * 512],
                            selb,
                            Phi_s[:, vc * 512:(vc + 1) * 512],
                            start=True, stop=False,
                        )
                    for vc in range(NV):
                        ins = pe.matmul(
                            psum[i % 4][:, vc * 512:(vc + 1) * 512],
                            selb,
                            Plo_s[:, vc * 512:(vc + 1) * 512],
                            start=False, stop=is_dve(i),
                        )
                    if not is_dve(i):
                        for vc in range(NV):
                            ins = pe.matmul(
                                psum[i % 4][:, vc * 512:(vc + 1) * 512],
                                ident_s[:],
                                E_s[:, vc * 512:(vc + 1) * 512],
                                start=False, stop=True,
                            )
                    ins.then_inc(pe_done, 1)

        @blk.vector
        def _(v):
            for k in range(repeat):
                # copies for the 8 transposes
                dsts = [(encT_s, kk) for kk in range(KD)] + [(predT_s, kk) for kk in range(KD)]
                for j, (dst, kk) in enumerate(dsts):
                    v.wait_ge(pe_prep, PREP * k + j + 1)
                    v.tensor_copy(dst[:, kk, :], psum[j % 2][:, 0:128]).then_inc(cp_sem, 1)
                for hb in range(KH):                             # e1t + bias b0
                    v.wait_ge(pe_prep, PREP * k + 9 + hb)
                    v.tensor_scalar_add(
                        e1t_s[:, hb, :], psum[2 + hb % 2][:, 0:128], b0t_s[:, hb:hb + 1]
                    ).then_inc(cp_sem, 1)
                for hb in range(KH):                             # p1t
                    v.wait_ge(pe_prep, PREP * k + 13 + hb)
                    v.tensor_copy(
                        p1t_s[:, hb, :], psum[2 + hb % 2][:, 512:640]
                    ).then_inc(cp_sem, 1)
                v.wait_ge(pe_prep, PREP * k + 17)
                v.tensor_copy(E_s[:], psum[0][:]).then_inc(cp_sem, 1)
                v.wait_ge(pe_prep, PREP * k + 18)
                v.tensor_copy(P_s[:], psum[1][:]).then_inc(cp_sem, 1)
                # exact bf16 split: Phi = bf16(P'), Plo = bf16(P' - fp32(Phi))
                v.tensor_copy(Phi_s[:], P_s[:]).then_inc(cp_sem, 1)       # 19
                v.tensor_copy(Pf_s[:], Phi_s[:]).then_inc(cp_sem, 1)      # 20
                v.tensor_sub(Plo_s[:], P_s[:], Pf_s[:]).then_inc(cp_sem, 1)  # 21
                # --- DVE share of phase B (add E' while draining PSUM)
                for i in range(ROWS):
                    if not is_dve(i):
                        continue
                    g_abs = NGROUPS * k + i // G
                    if i % G == 0 and g_abs >= 4:
                        # obuf[g%4] was last read by output-DMA group g_abs-4;
                        # 4-deep ring lets production run ~3 groups ahead of
                        # the drain, hiding the iteration-boundary phase A
                        v.wait_ge(dma_out, 16 * (g_abs - 3))
                    v.wait_ge(pe_done, PD * k + i + 1)
                    v.tensor_add(
                        obuf[(i // G) % 4][:, i % G, :], psum[i % 4][:], E_s[:]
                    ).then_inc(dve_done, 1)

    return nc


def _in_maps(pred_inp, enc_inp, W0, b0, W1, b1):
    maps = []
    for c in range(NCORES):
        b = c // 4
        t0 = (c % 4) * ROWS
        maps.append({
            "enc": np.ascontiguousarray(enc_inp[b, t0:t0 + ROWS, :], dtype=np.float32),
            "pred": np.ascontiguousarray(pred_inp[b], dtype=np.float32),
            "w0": np.ascontiguousarray(W0, dtype=np.float32),
            "w1": np.ascontiguousarray(W1, dtype=np.float32),
            "b0": np.ascontiguousarray(b0, dtype=np.float32),
            "b1": np.ascontiguousarray(b1, dtype=np.float32),
        })
    return maps


def _run(pred_inp, enc_inp, W0, b0, W1, b1, trace=False):
    from concourse.bass_utils import run_bass_kernel_spmd

    if "nc" not in _cache:
        _cache["nc"] = _build_nc(repeat=1)
    nc = _cache["nc"]
    res = run_bass_kernel_spmd(
        nc, _in_maps(pred_inp, enc_inp, W0, b0, W1, b1),
        list(range(NCORES)), trace=trace,
    )
    out = np.empty((B, T, U, V), dtype=np.float32)
    for c in range(NCORES):
        b = c // 4
        t0 = (c % 4) * ROWS
        out[b, t0:t0 + ROWS] = res.results[c]["out"]
    return out, res


def kernel(pred_inp, enc_inp, W0, b0, W1, b1):
    out, _ = _run(pred_inp, enc_inp, W0, b0, W1, b1, trace=False)
    return out


def _setup_timed(nc):
    """jit + fast-dispatch-compile the 8-core shard_map execution of `nc`."""
    import jax
    from concourse import bass2jax, mybir

    bass2jax.install_neuronx_cc_hook()
    in_names, out_names, out_avals, zero_outs = [], [], [], []
    pname = nc.partition_id_tensor.name if nc.partition_id_tensor else None
    for alloc in nc.m.functions[0].allocations:
        if not isinstance(alloc, mybir.MemoryLocationSet):
            continue
        name = alloc.memorylocations[0].name
        if alloc.kind == "ExternalInput":
            if name != pname:
                in_names.append(name)
        elif alloc.kind == "ExternalOutput":
            out_names.append(name)
            shape = tuple(alloc.tensor_shape)
            dt = mybir.dt.np(alloc.dtype)
            out_avals.append(jax.core.ShapedArray(shape, dt))
            zero_outs.append(np.zeros(shape, dt))
    n_params = len(in_names)
    all_names = in_names + out_names
    if pname is not None:
        all_names = all_names + [pname]

    def _body(*args):
        operands = list(args)
        if pname is not None:
            operands.append(bass2jax.partition_id_tensor())
        outs = bass2jax._bass_exec_p.bind(
            *operands,
            out_avals=tuple(out_avals),
            in_names=tuple(all_names),
            out_names=tuple(out_names),
            lowering_input_output_aliases=(),
            sim_require_finite=True,
            sim_require_nnan=True,
            nc=nc,
        )
        return tuple(outs)

    devices = jax.devices()[:NCORES]
    mesh = bass2jax.Mesh(np.asarray(devices), ("core",))
    P = bass2jax.PartitionSpec("core")
    donate = tuple(range(n_params, n_params + len(out_names)))
    jitted = jax.jit(
        bass2jax.shard_map(
            _body, mesh=mesh, in_specs=(P,) * (n_params + len(out_names)),
            out_specs=(P,) * len(out_names), check_rep=False,
        ),
        donate_argnums=donate, keep_unused=True,
    )
    sh = jax.sharding.NamedSharding(mesh, P)
    return jitted, in_names, zero_outs, sh


TIMED_REPEAT = 96     # kernel-body repetitions inside the timed NEFF
TIMED_CHAIN = 512     # executions chained per measured sync
TIMED_BATCHES = 3


def _timed_run(pred_inp, enc_inp, W0, b0, W1, b1, iters=None):
    """Steady-state on-device timing (no NTFF hook in this container).

    The timed NEFF contains TIMED_REPEAT back-to-back emissions of the full
    kernel body (loads + E'/P' + output streaming); TIMED_CHAIN executions of
    it are chained through donated output buffers and synced once. Reported
    time = wall / (TIMED_CHAIN * TIMED_REPEAT): the steady-state time of one
    full kernel execution on the hardware. A single non-amortized dispatch
    would instead measure the ~90 ms axon-tunnel round-trip, ~300x the
    kernel's actual device time.
    Returns (full_output, best_exec_ns).
    """
    import time
    import jax
    from concourse import bass2jax

    if "nc_t" not in _cache:
        _cache["nc_t"] = _build_nc(repeat=TIMED_REPEAT)
    nc = _cache["nc_t"]
    jitted, in_names, zero_outs, sh = _setup_timed(nc)

    maps = _in_maps(pred_inp, enc_inp, W0, b0, W1, b1)
    concat_in = [
        jax.device_put(
            np.concatenate([maps[c][nm] for c in range(NCORES)], axis=0), sh
        )
        for nm in in_names
    ]
    jax.block_until_ready(concat_in)
    d_zeros = [
        jax.device_put(
            np.zeros((NCORES * z.shape[0], *z.shape[1:]), z.dtype), sh
        )
        for z in zero_outs
    ]
    jax.block_until_ready(d_zeros)

    fast = bass2jax.fast_dispatch_compile(
        lambda: jitted.lower(*concat_in, *d_zeros).compile()
    )
    outs = fast(*concat_in, *d_zeros)
    jax.block_until_ready(outs)

    best = None
    for it in range(TIMED_BATCHES):
        t0 = time.perf_counter()
        for _ in range(TIMED_CHAIN):
            outs = fast(*concat_in, *outs)
        jax.block_until_ready(outs)
        dt_ns = (time.perf_counter() - t0) * 1e9 / (TIMED_CHAIN * TIMED_REPEAT)
        if os.environ.get("TIME_DEBUG"):
            print(f"  batch {it}: {dt_ns/1e3:.1f} us/exec")
        best = dt_ns if best is None else min(best, dt_ns)

    res0 = np.asarray(outs[0]).reshape(NCORES, ROWS, U, V)
    full = np.empty((B, T, U, V), dtype=np.float32)
    for c in range(NCORES):
        b = c // 4
        t0_ = (c % 4) * ROWS
        full[b, t0_:t0_ + ROWS] = res0[c]
    return full, int(best)
